# revision 3
# baseline (speedup 1.0000x reference)
"""AttentionBlock (GroupNorm + 4-head self-attention + proj + residual) on 8 TRN2 cores.

Data-parallel over batch: 16 batch elements -> 2 per NeuronCore; no collectives.

v2 additions (A/B-verified on hardware, same-window interleaved bench):
  - Cross-iteration pipelining: 12 x-pool slots give the next loop
    iteration's x loads fresh slots, and y stores issue from the ACT HWDGE
    queue (idle during proj), so the SP queue carries only x loads and the
    next iteration's x prefetch starts during attention. The serial
    norm+qkv prologue then hides under the previous iteration's tail.
  - S = K^T Q as fp8 zero-stride-pair DoubleRow: the pair axis reads the
    same K/Q bytes twice (doubling the score, folded into the exp scale),
    halving S-matmul PE time; Q/K tiles are fp8 (rel err 7.0e-3 vs 5.8e-3).
  - Schraudolph exps on kt 5,6,7 (3/unit instead of 2): with the S matmuls
    cheaper, attention is exp-bound, so more exp work shifts to DVE.

Design (per core, per batch element; C=512 channels, N=1024 tokens):
  - GroupNorm: bn_stats chains packed across all 4 channel chunks (one
    XOR-butterfly partition all-reduce on [128,8]); rstd = exp(-0.5*ln(var+eps))
    so the ACT engine only ever loads the natural_log_exp table set (no
    sqrt<->exp table reloads anywhere in the kernel); h is written straight to
    fp8e4 DoubleRow pair tiles [128, 2, 1024].
  - QKV: fp8e4 DoubleRow matmuls (contraction 256/matmul) with host-packed
    x16-scaled weight pairs; Q,K cast to fp16 [d,1024] tiles by ACT (bias
    fused); V written as [128, 2(nt-pair), 512] fp8 pair tiles by one DVE copy
    per psum pair. V-bias is folded into the proj bias on host (softmax rows
    sum to 1, so o = P(v+b) = Pv + b and proj(o+b) = proj(o) + W_p @ b).
  - Attention unit = (batch, head) over the full 1024-query range: S^T = K^T Q
    in fp16 (no PE transposes; two query-halves share a 2-bank PSUM tile; ONE
    exp per kt covers [128,1024]); softmax max-shift replaced by a constant
    shift (exp(s*scale - 2.5), inputs are bounded so this cannot overflow and
    keeps P inside fp8e4's +-240); P written directly to fp8 DoubleRow pair
    tiles. O^T = V P and the denominator run as fp8 DoubleRow over kt-pairs,
    interleaved with the next unit's S stream to keep PE dense. The
    denominator matmul uses lhsT = 16*ones[128,2,128], replicating 16*denom to
    all 128 PSUM rows: the tail is then just DVE reciprocal (which also folds
    the 1/16 weight-scale compensation) + one multiply -- no partition
    broadcast needed. Tails are deferred one unit.
  - proj: plain fp16 matmuls over fp16 O tiles; residual + bias fused in one
    scalar_tensor_tensor; x stays resident in SBUF from the norm phase (no
    reload for the residual). proj(0) is emitted in two halves straddling the
    last unit's tail so its PSUM allocations don't rotate into (and stall on)
    the flush pair's banks.

Environment workaround: this walrus build encodes at most one semaphore wait
per instruction; _split_multiwait() moves excess waits onto injected
same-engine NoOps after TileContext scheduling.
"""

import math

import numpy as np

import concourse.bass as bass
import concourse.mybir as mybir
from concourse import tile
from concourse.bass_utils import run_bass_kernel_spmd

# problem constants (self-contained by contract)
B, C, H, W = 16, 512, 32, 32
N = H * W
HEADS, D = 4, 128
G = 32
EPS = 1e-5
SCALE = 1.0 / math.sqrt(D)
NCORES = 8
NB = B // NCORES  # batch elems per core
CK = C // 128     # channel chunks
NT = N // 128     # token tiles
F32 = mybir.dt.float32
F16 = mybir.dt.float16
F8 = mybir.dt.float8e4
I32 = mybir.dt.int32
U8 = mybir.dt.uint8
FX = mybir.ActivationFunctionType
ALU = mybir.AluOpType
DR = mybir.MatmulPerfMode.DoubleRow

WARM_MMS = 24  # PE warmup matmuls issued during the groupnorm prologue
WSCALE = 16.0   # qkv weights are host-scaled by 16 for fp8e4 range coverage
ESHIFT = 2.5    # global exp shift: keeps P = exp(s - ESHIFT) under fp8e4 +-240


# --- workaround: this walrus encodes at most ONE sync wait per instruction ---
_waitctr = [0]


def _split_multiwait(nc):
    for fn in nc.m.functions:
        for bb in fn.blocks:
            out = []
            changed = False
            for inst in bb.instructions:
                si = inst.sync_info
                if si is not None and len(si.on_wait) > 1:
                    waits = list(si.on_wait)
                    for wt in waits[:-1]:
                        _waitctr[0] += 1
                        nop = mybir.InstNoOp(
                            name=f"I-waitsplit-{_waitctr[0]}", ins=[], outs=[]
                        )
                        nop.engine = inst.engine
                        nop.sync_info = mybir.SyncInfo(on_wait=[wt], on_update=[])
                        out.append(nop)
                    inst.sync_info = mybir.SyncInfo(
                        on_wait=[waits[-1]], on_update=list(si.on_update)
                    )
                    changed = True
                out.append(inst)
            if changed:
                bb.instructions = out


def _build(loop_n=None, ablate=()):
    ab = set(ablate)
    s_fp8 = "no_sfp8" not in ab
    QKDT = F8 if s_fp8 else F16
    nc = bass.Bass("TRN2", target_bir_lowering=False, debug=False)

    x_d = nc.dram_tensor("x", [NB, C, N], F32, kind="ExternalInput")
    qw8_d = nc.dram_tensor("qw8", [2, 128, 2, 3 * C], F8, kind="ExternalInput")
    qb_d = nc.dram_tensor("qb", [3 * C], F32, kind="ExternalInput")
    pwT_d = nc.dram_tensor("pwT", [C, C], F16, kind="ExternalInput")
    pb_d = nc.dram_tensor("pb", [C], F32, kind="ExternalInput")
    nw_d = nc.dram_tensor("nw", [C], F32, kind="ExternalInput")
    nb_d = nc.dram_tensor("nb", [C], F32, kind="ExternalInput")
    y_d = nc.dram_tensor("y", [NB, C, N], F32, kind="ExternalOutput")

    with tile.TileContext(nc) as tc:
        import contextlib

        with contextlib.ExitStack() as ctx:
            wpool = ctx.enter_context(tc.tile_pool(name="wpool", bufs=1))
            # 12 x slots (8/iter used): the next loop iteration's x loads get
            # fresh slots, so they prefetch under this iteration's attention
            # instead of waiting for proj's residual reads to free x
            xpool = ctx.enter_context(tc.tile_pool(name="xpool", bufs=12))
            spool = ctx.enter_context(tc.tile_pool(name="spool", bufs=3))
            stp = ctx.enter_context(tc.tile_pool(name="stp", bufs=3))
            hpool = ctx.enter_context(tc.tile_pool(name="hpool", bufs=8))
            qkpool = ctx.enter_context(tc.tile_pool(name="qkpool", bufs=17))
            vpool = ctx.enter_context(tc.tile_pool(name="vpool", bufs=9))
            ppool = ctx.enter_context(tc.tile_pool(name="ppool", bufs=9))
            opool = ctx.enter_context(tc.tile_pool(name="opool", bufs=9))
            ypool = ctx.enter_context(tc.tile_pool(name="ypool", bufs=2))
            psb = ctx.enter_context(tc.tile_pool(name="psb", bufs=4, space="PSUM"))

            # ---- constants & weights ----
            ones_col = wpool.tile([128, 1], F16, tag="ones_col")
            nc.gpsimd.memset(ones_col[:], 1.0)
            # dn lhsT: pair-axis step must be 16B-aligned (s3_lw dual-fp8 rule)
            # dn lhsT: 16*ones, M=128 -> dn replicated to all PSUM rows; the 16
            # also folds the 1/WSCALE v-compensation into the reciprocal.
            ones8 = wpool.tile([128, 2, 128], F8, tag="ones8")
            nc.gpsimd.memset(ones8[:], WSCALE)
            ones_w = wpool.tile([128, 512], F16, tag="ones_w")
            nc.gpsimd.memset(ones_w[:], 1.0)
            eps128 = wpool.tile([128, 1], F32, tag="eps")
            nc.gpsimd.memset(eps128[:], EPS)
            eshift = wpool.tile([128, 1], F32, tag="eshift")
            nc.gpsimd.memset(eshift[:], -ESHIFT)

            # PE warmup (opt-in): drift-free A/B showed the warmup matmuls
            # cost ~30us/iter in the repeat-loop regime, so default is off.
            warm_ps = psb.tile([128, 1024], F32, tag="ps", name="warmps")
            for _ in range(WARM_MMS if "warm" in ab else 0):
                nc.tensor.matmul(
                    warm_ps[0:1, 0:512], ones_col[:], ones_w[:], start=True, stop=True
                )

            wq8 = []
            for pr in range(2):
                t = wpool.tile([128, 2, 3 * C], F8, tag=f"wq8_{pr}")
                nc.sync.dma_start(t[:], qw8_d.ap()[pr, :, :, :])
                wq8.append(t)
            wp = []
            for ck in range(CK):
                t = wpool.tile([128, C], F16, tag=f"wp{ck}")
                nc.sync.dma_start(t[:], pwT_d.ap()[128 * ck : 128 * (ck + 1), :])
                wp.append(t)
            qbv = wpool.tile([128, 12], F32, tag="qbv")
            nc.sync.dma_start(qbv[:], qb_d.ap()[:].rearrange("(c p) -> p c", p=128))
            pbv = wpool.tile([128, CK], F32, tag="pbv")
            nc.sync.dma_start(pbv[:], pb_d.ap()[:].rearrange("(c p) -> p c", p=128))
            nwv = wpool.tile([128, CK], F32, tag="nwv")
            nc.sync.dma_start(nwv[:], nw_d.ap()[:].rearrange("(c p) -> p c", p=128))
            nbv = wpool.tile([128, CK], F32, tag="nbv")
            nc.sync.dma_start(nbv[:], nb_d.ap()[:].rearrange("(c p) -> p c", p=128))

            # x loads split per token-half so bn_stats can start at half-tile
            def emit_x_dmas(b):
                xts = []
                for ck in range(CK):
                    xt = xpool.tile([128, N], F32, tag="x", name=f"xt{b}_{ck}")
                    for hf in range(2):
                        nc.sync.dma_start(
                            xt[:, 512 * hf : 512 * (hf + 1)],
                            x_d.ap()[
                                b, 128 * ck : 128 * (ck + 1),
                                512 * hf : 512 * (hf + 1),
                            ],
                        )
                    xts.append(xt)
                return xts

            # ---- groupnorm: packed stats across the 4 channel chunks ----
            def emit_norm(b, xts):
                st48 = stp.tile([128, 48], F32, tag="st48", name=f"st48_{b}")
                for ck in range(CK):
                    nc.vector.bn_stats(
                        st48[:, 12 * ck : 12 * ck + 6], xts[ck][:, 0:512]
                    )
                    nc.vector.bn_stats(
                        st48[:, 12 * ck + 6 : 12 * ck + 12], xts[ck][:, 512:1024]
                    )
                # view [p, c, f, t]: c=4 chunks, f=4 subgroups(256 tok), t=(cnt,mean,M2)
                st4 = st48[:, :].rearrange("p (c f t) -> p c f t", c=4, f=4, t=3)
                sum_m = stp.tile([128, CK, 1], F32, tag="sum_m")
                nc.vector.tensor_reduce(
                    sum_m[:], st4[:, :, :, 1], mybir.AxisListType.X, ALU.add
                )
                msq = stp.tile([128, CK, 4], F32, tag="msq")
                nc.vector.tensor_mul(msq[:], st4[:, :, :, 1], st4[:, :, :, 1])
                sum_m2 = stp.tile([128, CK, 1], F32, tag="sum_m2")
                nc.vector.tensor_reduce(
                    sum_m2[:], msq[:], mybir.AxisListType.X, ALU.add
                )
                sum_cv = stp.tile([128, CK, 1], F32, tag="sum_cv")
                nc.vector.tensor_reduce(
                    sum_cv[:], st4[:, :, :, 2], mybir.AxisListType.X, ALU.add
                )
                # sg8 packed [p, (c s)]: s0=sum(x), s1=sum(x^2) per channel
                sg = stp.tile([128, 2 * CK], F32, tag="sg", name=f"sg{b}")
                sgv = sg[:, :].rearrange("p (c s) -> p c s", s=2)
                nc.vector.tensor_scalar(
                    sgv[:, :, 0:1], sum_m[:], 256.0, None, ALU.mult
                )
                nc.vector.scalar_tensor_tensor(
                    sgv[:, :, 1:2], sum_m2[:], 256.0, sum_cv[:], ALU.mult, ALU.add
                )
                # XOR-butterfly partition all-reduce within each 16-channel group
                for k in (1, 2, 4, 8):
                    tmp = stp.tile([128, 2 * CK], F32, tag="shuf")
                    nc.vector.stream_shuffle(
                        tmp[:], sg[:], [i ^ k for i in range(32)]
                    )
                    sg2 = stp.tile([128, 2 * CK], F32, tag="sg", name=f"sg{b}_{k}")
                    nc.vector.tensor_add(sg2[:], sg[:], tmp[:])
                    sg = sg2
                sgv = sg[:, :].rearrange("p (c s) -> p c s", s=2)
                mean = stp.tile([128, CK, 1], F32, tag="mean")
                nc.vector.tensor_scalar(
                    mean[:], sgv[:, :, 0:1], 1.0 / 16384.0, None, ALU.mult
                )
                e2 = stp.tile([128, CK, 1], F32, tag="e2")
                nc.vector.tensor_scalar(
                    e2[:], sgv[:, :, 1:2], 1.0 / 16384.0, None, ALU.mult
                )
                msq2 = stp.tile([128, CK, 1], F32, tag="msq2")
                nc.vector.tensor_mul(msq2[:], mean[:], mean[:])
                var = stp.tile([128, CK], F32, tag="var")
                varv = var[:, :].rearrange("p (c o) -> p c o", o=1)
                nc.vector.tensor_sub(varv[:], e2[:], msq2[:])
                # rstd = exp(-0.5*ln(var+eps)) -- keeps ACT on the ln/exp table set
                lnv = stp.tile([128, CK], F32, tag="lnv")
                nc.scalar.activation(lnv[:], var[:], FX.Ln, bias=eps128[:, 0:1])
                rstd = stp.tile([128, CK], F32, tag="rstd")
                nc.scalar.activation(rstd[:], lnv[:], FX.Exp, scale=-0.5)
                svec = stp.tile([128, CK], F32, tag="svec", name=f"svec{b}")
                nc.vector.tensor_mul(svec[:], nwv[:], rstd[:])
                tms = stp.tile([128, CK], F32, tag="tms")
                nc.vector.tensor_mul(tms[:], mean[:, :, 0], svec[:])
                tvec = stp.tile([128, CK], F32, tag="tvec", name=f"tvec{b}")
                nc.vector.tensor_sub(tvec[:], nbv[:], tms[:])
                hs = [
                    hpool.tile([128, 2, N], F8, tag="h", name=f"h{b}_{pr}")
                    for pr in range(2)
                ]
                for ck in range(CK):
                    dst = hs[ck // 2][:, ck % 2, :]
                    # h-writes are SBUF->SBUF so the Pool engine could take
                    # them ("pool_h"), but measured A/B showed real Pool ops
                    # run below the cost model's estimate: default DVE
                    eng = nc.gpsimd if "pool_h" in ab else nc.vector
                    eng.tensor_scalar(
                        dst, xts[ck][:], svec[:, ck : ck + 1],
                        tvec[:, ck : ck + 1], ALU.mult, ALU.add,
                    )
                return hs

            def emit_qkv(b, hs):
                """QK as [d,1024] per head-slice tile; V as [128,(nt-pair,C)].
                All matmuls fp8 DoubleRow over ck-pairs (contraction 256/MM)."""
                qk = []
                for t8 in range(8):
                    ps = psb.tile([128, 1024], F32, tag="ps", name=f"qkps{b}_{t8}")
                    for qh in range(2):
                        for pr in range(2):
                            nc.tensor.matmul(
                                ps[:, 512 * qh : 512 * (qh + 1)],
                                wq8[pr][:, :, 128 * t8 : 128 * (t8 + 1)],
                                hs[pr][:, :, 512 * qh : 512 * (qh + 1)],
                                start=(pr == 0),
                                stop=(pr == 1),
                                perf_mode=DR,
                            )
                    sb = qkpool.tile([128, N], QKDT, tag="qk")
                    nc.scalar.activation(
                        sb[:], ps[:], FX.Identity, bias=qbv[:, t8 : t8 + 1]
                    )
                    qk.append(sb)
                vts = []
                for np_ in range(NT // 2):
                    ps = psb.tile([128, 1024], F32, tag="ps", name=f"vps{b}_{np_}")
                    for half in range(2):
                        nt = 2 * np_ + half
                        for pr in range(2):
                            nc.tensor.matmul(
                                ps[:, 512 * half : 512 * (half + 1)],
                                hs[pr][:, :, 128 * nt : 128 * (nt + 1)],
                                wq8[pr][:, :, 2 * C : 3 * C],
                                start=(pr == 0),
                                stop=(pr == 1),
                                perf_mode=DR,
                            )
                    vt = vpool.tile([128, 2, C], F8, tag="v")
                    nc.vector.tensor_copy(
                        vt[:, :, :].rearrange("p a b -> p (a b)"), ps[:]
                    )
                    vts.append(vt)
                return qk, vts

            # attention unit = (batch, head) over all 1024 queries.
            # prev unit's O/dn matmuls interleave with this unit's S/exp stream.
            # O and dn are fp8 DoubleRow over kt-pairs (V and P are pair tiles).
            def emit_prev_mms(prev, t, vts_by_b):
                pb_, ph, pptiles, dn_ps, ot_ps = prev
                NP2 = NT // 2
                for qh in range(2):
                    nc.tensor.matmul(
                        ot_ps[:, 512 * qh : 512 * (qh + 1)],
                        vts_by_b[pb_][t][:, :, 128 * ph : 128 * (ph + 1)],
                        pptiles[t][:, :, 512 * qh : 512 * (qh + 1)],
                        start=(t == 0),
                        stop=(t == NP2 - 1),
                        perf_mode=DR,
                    )
                if "no_dn" not in ab:
                    for qh in range(2):
                        nc.tensor.matmul(
                            dn_ps[:, 512 * qh : 512 * (qh + 1)],
                            ones8[:, :, :],
                            pptiles[t][:, :, 512 * qh : 512 * (qh + 1)],
                            start=(t == 0),
                            stop=(t == NP2 - 1),
                            perf_mode=DR,
                        )

            # fp8 S: zero-stride pair axis reads the same K/Q bytes for both
            # DoubleRow contraction rows, doubling the score (folded into
            # ESC) while halving PE time per S matmul
            ESC = SCALE / (WSCALE * WSCALE) / (2.0 if s_fp8 else 1.0)
            # Schraudolph exp evaluated straight into fp8e4 BITS on DVE:
            # uint8(round(log2(e^x) * 8 + 56)) IS the fp8 encoding of ~e^x,
            # at fp8's own quantization granularity. One tensor_scalar per
            # tile offloads the ACT engine (the attention-phase bottleneck);
            # the denominator (summed from the same quantized P) absorbs the
            # sawtooth error (numpy-validated: rel err unchanged). Applied to
            # the LAST kt pair only -- those P tiles are consumed latest by
            # the next unit's O/dn matmuls, so the DVE op is off the PE path.
            if "no_schr" in ab:
                SCHR_KT = ()
            elif "schr4" in ab:
                SCHR_KT = (4, 5, 6, 7)
            elif "schr2" in ab:
                SCHR_KT = (6, 7)
            else:
                # with fp8 S the attention phase is exp-bound, not PE-bound:
                # 3 of 8 exps per unit go to DVE as Schraudolph
                SCHR_KT = (5, 6, 7)
            SCHR_A8 = 8.0 * ESC / math.log(2.0)
            SCHR_B8 = 56.0 - 8.0 * ESHIFT / math.log(2.0)

            def emit_unit(b, h, qk_by_b, vts_by_b, prev):
                """Returns (cur, prev_with_psums): prev gets its dn/ot PSUM
                tiles allocated here so its matmuls interleave with cur's."""
                q_sb, k_sb = qk_by_b[b][h], qk_by_b[b][HEADS + h]
                ptiles = [
                    ppool.tile([128, 2, 1024], F8, tag="p", name=f"p{b}_{h}_{t}")
                    for t in range(NT // 2)
                ]
                if prev is not None:
                    dn_ps = None
                    if "no_dn" not in ab:
                        dn_ps = psb.tile([128, 1024], F32, tag="ps", name=f"dn{b}_{h}")
                    ot_ps = psb.tile([128, 1024], F32, tag="ps", name=f"ot{b}_{h}")
                    prev = prev[:3] + (dn_ps, ot_ps)
                for kt in range(NT):
                    if prev is not None and kt % 2 == 0:
                        emit_prev_mms(prev, kt // 2, vts_by_b)
                    s_ps = psb.tile(
                        [128, 1024], F32, tag="ps", name=f"s{b}_{h}_{kt}"
                    )
                    if s_fp8:
                        k_sl = k_sb[:, 128 * kt : 128 * (kt + 1)].rearrange(
                            "p (o m) -> p o m", o=1
                        ).broadcast_to([128, 2, 128])
                        for qh in range(2):
                            q_sl = q_sb[:, 512 * qh : 512 * (qh + 1)].rearrange(
                                "p (o m) -> p o m", o=1
                            ).broadcast_to([128, 2, 512])
                            nc.tensor.matmul(
                                s_ps[:, 512 * qh : 512 * (qh + 1)],
                                k_sl, q_sl, start=True, stop=True,
                                perf_mode=DR,
                            )
                    else:
                        for qh in range(2):
                            nc.tensor.matmul(
                                s_ps[:, 512 * qh : 512 * (qh + 1)],
                                k_sb[:, 128 * kt : 128 * (kt + 1)],
                                q_sb[:, 512 * qh : 512 * (qh + 1)],
                                start=True,
                                stop=True,
                            )
                    dst = ptiles[kt // 2][:, kt % 2, :]
                    if kt in SCHR_KT:
                        nc.vector.tensor_scalar(
                            dst.bitcast(U8), s_ps[:], SCHR_A8, SCHR_B8,
                            ALU.mult, ALU.add,
                        )
                    else:
                        nc.scalar.activation(
                            dst, s_ps[:], FX.Exp, bias=eshift[:, 0:1], scale=ESC
                        )
                return (b, h, ptiles, None, None), prev

            def emit_unit_tail(prev, osb_by_b):
                pb_, ph, pptiles, dn_ps, ot_ps = prev
                osb = osb_by_b[pb_][ph]
                if "no_dn" in ab:
                    nc.vector.tensor_copy(osb[:], ot_ps[:])
                    return
                # dn_ps rows all hold 16*denom (ones8=16 replicated matmul);
                # reciprocal folds the 1/16 v-scale compensation for free
                R_sb = spool.tile([128, 1024], F16, tag="Rsb")
                with nc.allow_low_precision(
                    reason="r in [1e-4,1e-2]: fp16 normal range, 0.1% rel err"
                ):
                    nc.vector.reciprocal(R_sb[:], dn_ps[:])
                nc.vector.tensor_mul(osb[:], ot_ps[:], R_sb[:])

            def emit_attn(qk_by_b, vts_by_b):
                osb_by_b = {
                    bb: [
                        opool.tile([128, N], F16, tag="o", name=f"osb{bb}_{i}")
                        for i in range(HEADS)
                    ]
                    for bb in (0, 1)
                }
                units = []
                for h in range(HEADS):
                    units.append((0, h))
                    units.append((1, h))
                prev = None
                pending = []
                for bb, h in units:
                    cur, prev_upd = emit_unit(bb, h, qk_by_b, vts_by_b, prev)
                    if prev_upd is not None:
                        pending.append(prev_upd)
                    while len(pending) > 1:
                        emit_unit_tail(pending.pop(0), osb_by_b)
                    prev = cur
                # last unit's O/dn matmuls (its tail is emitted by schedule()
                # between the two proj phases so proj(0) overlaps it)
                if prev is not None:
                    if "no_dn" not in ab:
                        dn_ps = psb.tile([128, 1024], F32, tag="ps", name="dnF")
                    else:
                        dn_ps = None
                    ot_ps = psb.tile([128, 1024], F32, tag="ps", name="otF")
                    prev = prev[:3] + (dn_ps, ot_ps)
                if pending:
                    emit_unit_tail(pending.pop(0), osb_by_b)
                if prev is not None:
                    for t in range(NT // 2):
                        emit_prev_mms(prev, t, vts_by_b)
                return osb_by_b, prev

            def emit_proj(b, osb, xts, t4s=range(CK)):
                for t4 in t4s:
                    ps = psb.tile([128, 1024], F32, tag="ps", name=f"prps{b}_{t4}")
                    for qh in range(2):
                        for hh in range(HEADS):
                            nc.tensor.matmul(
                                ps[:, 512 * qh : 512 * (qh + 1)],
                                wp[hh][:, 128 * t4 : 128 * (t4 + 1)],
                                osb[hh][:, 512 * qh : 512 * (qh + 1)],
                                start=(hh == 0),
                                stop=(hh == HEADS - 1),
                            )
                    yt = ypool.tile([128, N], F32, tag="y")
                    nc.vector.scalar_tensor_tensor(
                        yt[:], ps[:], pbv[:, t4 : t4 + 1], xts[t4][:],
                        ALU.add, ALU.add,
                    )
                    # y stores issue from the ACT HWDGE queue (idle during
                    # proj) so the SP queue carries only x loads -- the next
                    # iteration's x prefetch then issues during attention
                    # instead of after the last y store
                    nc.scalar.dma_start(
                        y_d.ap()[b, 128 * t4 : 128 * (t4 + 1), :], yt[:]
                    )

            # ---- schedule ----
            def schedule():
                xts0 = emit_x_dmas(0)
                hs0 = emit_norm(0, xts0)
                qk0, vts0 = emit_qkv(0, hs0)
                xts1 = emit_x_dmas(1)
                hs1 = emit_norm(1, xts1)
                qk1, vts1 = emit_qkv(1, hs1)
                vts_by_b = {0: vts0, 1: vts1}
                qk_by_b = {0: qk0, 1: qk1}
                osb_by_b, last = emit_attn(qk_by_b, vts_by_b)
                # proj(0) halves straddle the last tail: its later PSUM
                # allocations would otherwise rotate into the flush pair's
                # banks and stall until the tail frees them
                emit_proj(0, osb_by_b[0], xts0, range(0, 2))
                if last is not None:
                    emit_unit_tail(last, osb_by_b)
                emit_proj(0, osb_by_b[0], xts0, range(2, CK))
                emit_proj(1, osb_by_b[1], xts1)

            if loop_n is None:
                schedule()
            elif loop_n < 0:
                # python-unrolled repeat (for TimelineSim steady-state runs)
                for _ in range(-loop_n):
                    schedule()
            else:
                with tc.For_i(0, loop_n, 1):
                    schedule()

    _split_multiwait(nc)
    return nc


_CACHE = {}


def _get_program(loop_n=None, ablate=()):
    key = ("nc", loop_n, tuple(sorted(ablate)))
    if key not in _CACHE:
        _CACHE[key] = _build(loop_n, ablate)
    return _CACHE[key]


def _make_in_maps(inputs):
    x = np.ascontiguousarray(np.asarray(inputs["x"], dtype=np.float32))
    qkv_w = np.asarray(inputs["qkv_w"], dtype=np.float32)
    qkv_b = np.ascontiguousarray(np.asarray(inputs["qkv_b"], dtype=np.float32))
    proj_w = np.asarray(inputs["proj_w"], dtype=np.float32)
    proj_b = np.ascontiguousarray(np.asarray(inputs["proj_b"], dtype=np.float32))
    norm_w = np.ascontiguousarray(np.asarray(inputs["norm_w"], dtype=np.float32))
    norm_b = np.ascontiguousarray(np.asarray(inputs["norm_b"], dtype=np.float32))
    import ml_dtypes

    # qkv weights: x16 scale, fp8e4, DoubleRow pair layout [pair, ki, ko, out]
    qwT = qkv_w.T * WSCALE                      # [C, 3C]
    qw8 = np.ascontiguousarray(
        qwT.reshape(2, 2, 128, 3 * C).transpose(0, 2, 1, 3)
    ).astype(ml_dtypes.float8_e4m3)
    pwT = np.ascontiguousarray(proj_w.T.astype(np.float16))
    # v-bias folds through proj (softmax rows sum to 1): pb_eff = pb + Wp @ bv
    pb_eff = np.ascontiguousarray(
        proj_b + proj_w @ qkv_b[2 * C : 3 * C]
    ).astype(np.float32)
    # q/k biases ride the x16 weight scale (v-bias slot unused on device)
    qb_s = qkv_b.copy()
    qb_s[: 2 * C] *= WSCALE
    xs = x.reshape(NCORES, NB, C, N)
    in_maps = []
    for i in range(NCORES):
        in_maps.append(
            {
                "x": np.ascontiguousarray(xs[i]),
                "qw8": qw8.view(np.uint8),
                "qb": qb_s,
                "pwT": pwT,
                "pb": pb_eff,
                "nw": norm_w,
                "nb": norm_b,
            }
        )
    return in_maps


def _run(inputs, trace=False, loop_n=None, ablate=()):
    nc = _get_program(loop_n, ablate)
    in_maps = _make_in_maps(inputs)
    res = run_bass_kernel_spmd(
        nc, in_maps, core_ids=list(range(NCORES)), trace=trace
    )
    y = np.stack([res.results[i]["y"] for i in range(NCORES)], axis=0)
    y = y.reshape(B, C, H, W)
    return y, res


def kernel(**inputs) -> np.ndarray:
    y, _ = _run(inputs, trace=False)
    return y



# revision 4
# speedup vs baseline: 1.0350x; 1.0350x over previous
"""AttentionBlock (GroupNorm + 4-head self-attention + proj + residual) on 8 TRN2 cores.

Data-parallel over batch: 16 batch elements -> 2 per NeuronCore; no collectives.

Design (per core, per batch element; C=512 channels, N=1024 tokens):
  - GroupNorm: bn_stats chains packed across all 4 channel chunks (one
    XOR-butterfly partition all-reduce on [128,8]); rstd = exp(-0.5*ln(var+eps))
    so the ACT engine only ever loads the natural_log_exp table set (no
    sqrt<->exp table reloads anywhere in the kernel); h is written straight to
    fp8e4 DoubleRow pair tiles [128, 2, 1024].
  - QKV: fp8e4 DoubleRow matmuls (contraction 256/matmul) with host-packed
    x16-scaled weight pairs; Q,K cast to fp16 [d,1024] tiles by ACT (bias
    fused); V written as [128, 2(nt-pair), 512] fp8 pair tiles by one DVE copy
    per psum pair. V-bias is folded into the proj bias on host (softmax rows
    sum to 1, so o = P(v+b) = Pv + b and proj(o+b) = proj(o) + W_p @ b).
  - Attention unit = (batch, head) over the full 1024-query range: S^T = K^T Q
    in fp16 (no PE transposes; two query-halves share a 2-bank PSUM tile; ONE
    exp per kt covers [128,1024]); softmax max-shift replaced by a constant
    shift (exp(s*scale - 2.5), inputs are bounded so this cannot overflow and
    keeps P inside fp8e4's +-240); P written directly to fp8 DoubleRow pair
    tiles. O^T = V P and the denominator run as fp8 DoubleRow over kt-pairs,
    interleaved with the next unit's S stream to keep PE dense. The
    denominator matmul uses lhsT = 16*ones[128,2,128], replicating 16*denom to
    all 128 PSUM rows: the tail is then just DVE reciprocal (which also folds
    the 1/16 weight-scale compensation) + one multiply -- no partition
    broadcast needed. Tails are deferred one unit.
  - proj: plain fp16 matmuls over fp16 O tiles; residual + bias fused in one
    scalar_tensor_tensor; x stays resident in SBUF from the norm phase (no
    reload for the residual). proj(0) is emitted in two halves straddling the
    last unit's tail so its PSUM allocations don't rotate into (and stall on)
    the flush pair's banks.

Environment workaround: this walrus build encodes at most one semaphore wait
per instruction; _split_multiwait() moves excess waits onto injected
same-engine NoOps after TileContext scheduling.
"""

import math

import numpy as np

import concourse.bass as bass
import concourse.mybir as mybir
from concourse import tile
from concourse.bass_utils import run_bass_kernel_spmd

# problem constants (self-contained by contract)
B, C, H, W = 16, 512, 32, 32
N = H * W
HEADS, D = 4, 128
G = 32
EPS = 1e-5
SCALE = 1.0 / math.sqrt(D)
NCORES = 8
NB = B // NCORES  # batch elems per core
CK = C // 128     # channel chunks
NT = N // 128     # token tiles
F32 = mybir.dt.float32
F16 = mybir.dt.float16
F8 = mybir.dt.float8e4
I32 = mybir.dt.int32
U8 = mybir.dt.uint8
FX = mybir.ActivationFunctionType
ALU = mybir.AluOpType
DR = mybir.MatmulPerfMode.DoubleRow

WARM_MMS = 24  # PE warmup matmuls issued during the groupnorm prologue
WSCALE = 16.0   # qkv weights are host-scaled by 16 for fp8e4 range coverage
ESHIFT = 2.5    # global exp shift: keeps P = exp(s - ESHIFT) under fp8e4 +-240


# --- workaround: this walrus encodes at most ONE sync wait per instruction ---
_waitctr = [0]


def _split_multiwait(nc):
    for fn in nc.m.functions:
        for bb in fn.blocks:
            out = []
            changed = False
            for inst in bb.instructions:
                si = inst.sync_info
                if si is not None and len(si.on_wait) > 1:
                    waits = list(si.on_wait)
                    for wt in waits[:-1]:
                        _waitctr[0] += 1
                        nop = mybir.InstNoOp(
                            name=f"I-waitsplit-{_waitctr[0]}", ins=[], outs=[]
                        )
                        nop.engine = inst.engine
                        nop.sync_info = mybir.SyncInfo(on_wait=[wt], on_update=[])
                        out.append(nop)
                    inst.sync_info = mybir.SyncInfo(
                        on_wait=[waits[-1]], on_update=list(si.on_update)
                    )
                    changed = True
                out.append(inst)
            if changed:
                bb.instructions = out


def _build(loop_n=None, ablate=()):
    ab = set(ablate)
    nc = bass.Bass("TRN2", target_bir_lowering=False, debug=False)

    x_d = nc.dram_tensor("x", [NB, C, N], F32, kind="ExternalInput")
    qw8_d = nc.dram_tensor("qw8", [2, 128, 2, 3 * C], F8, kind="ExternalInput")
    qb_d = nc.dram_tensor("qb", [3 * C], F32, kind="ExternalInput")
    pwT_d = nc.dram_tensor("pwT", [C, C], F16, kind="ExternalInput")
    pb_d = nc.dram_tensor("pb", [C], F32, kind="ExternalInput")
    nw_d = nc.dram_tensor("nw", [C], F32, kind="ExternalInput")
    nb_d = nc.dram_tensor("nb", [C], F32, kind="ExternalInput")
    y_d = nc.dram_tensor("y", [NB, C, N], F32, kind="ExternalOutput")

    with tile.TileContext(nc) as tc:
        import contextlib

        with contextlib.ExitStack() as ctx:
            wpool = ctx.enter_context(tc.tile_pool(name="wpool", bufs=1))
            xpool = ctx.enter_context(tc.tile_pool(name="xpool", bufs=8))
            spool = ctx.enter_context(tc.tile_pool(name="spool", bufs=3))
            stp = ctx.enter_context(tc.tile_pool(name="stp", bufs=3))
            hpool = ctx.enter_context(tc.tile_pool(name="hpool", bufs=8))
            qkpool = ctx.enter_context(tc.tile_pool(name="qkpool", bufs=17))
            vpool = ctx.enter_context(tc.tile_pool(name="vpool", bufs=9))
            ppool = ctx.enter_context(tc.tile_pool(name="ppool", bufs=9))
            opool = ctx.enter_context(tc.tile_pool(name="opool", bufs=9))
            ypool = ctx.enter_context(tc.tile_pool(name="ypool", bufs=2))
            psb = ctx.enter_context(tc.tile_pool(name="psb", bufs=4, space="PSUM"))

            # ---- constants & weights ----
            ones_col = wpool.tile([128, 1], F16, tag="ones_col")
            nc.gpsimd.memset(ones_col[:], 1.0)
            # dn lhsT: pair-axis step must be 16B-aligned (s3_lw dual-fp8 rule)
            # dn lhsT: 16*ones, M=128 -> dn replicated to all PSUM rows; the 16
            # also folds the 1/WSCALE v-compensation into the reciprocal.
            ones8 = wpool.tile([128, 2, 128], F8, tag="ones8")
            nc.gpsimd.memset(ones8[:], WSCALE)
            ones_w = wpool.tile([128, 512], F16, tag="ones_w")
            nc.gpsimd.memset(ones_w[:], 1.0)
            eps128 = wpool.tile([128, 1], F32, tag="eps")
            nc.gpsimd.memset(eps128[:], EPS)
            eshift = wpool.tile([128, 1], F32, tag="eshift")
            nc.gpsimd.memset(eshift[:], -ESHIFT)

            # PE warmup (opt-in): drift-free A/B showed the warmup matmuls
            # cost ~30us/iter in the repeat-loop regime, so default is off.
            warm_ps = psb.tile([128, 1024], F32, tag="ps", name="warmps")
            for _ in range(WARM_MMS if "warm" in ab else 0):
                nc.tensor.matmul(
                    warm_ps[0:1, 0:512], ones_col[:], ones_w[:], start=True, stop=True
                )

            wq8 = []
            for pr in range(2):
                t = wpool.tile([128, 2, 3 * C], F8, tag=f"wq8_{pr}")
                nc.sync.dma_start(t[:], qw8_d.ap()[pr, :, :, :])
                wq8.append(t)
            wp = []
            for ck in range(CK):
                t = wpool.tile([128, C], F16, tag=f"wp{ck}")
                nc.sync.dma_start(t[:], pwT_d.ap()[128 * ck : 128 * (ck + 1), :])
                wp.append(t)
            qbv = wpool.tile([128, 12], F32, tag="qbv")
            nc.sync.dma_start(qbv[:], qb_d.ap()[:].rearrange("(c p) -> p c", p=128))
            pbv = wpool.tile([128, CK], F32, tag="pbv")
            nc.sync.dma_start(pbv[:], pb_d.ap()[:].rearrange("(c p) -> p c", p=128))
            nwv = wpool.tile([128, CK], F32, tag="nwv")
            nc.sync.dma_start(nwv[:], nw_d.ap()[:].rearrange("(c p) -> p c", p=128))
            nbv = wpool.tile([128, CK], F32, tag="nbv")
            nc.sync.dma_start(nbv[:], nb_d.ap()[:].rearrange("(c p) -> p c", p=128))

            # x loads split per token-half so bn_stats can start at half-tile
            def emit_x_dmas(b):
                xts = []
                for ck in range(CK):
                    xt = xpool.tile([128, N], F32, tag="x", name=f"xt{b}_{ck}")
                    for hf in range(2):
                        nc.sync.dma_start(
                            xt[:, 512 * hf : 512 * (hf + 1)],
                            x_d.ap()[
                                b, 128 * ck : 128 * (ck + 1),
                                512 * hf : 512 * (hf + 1),
                            ],
                        )
                    xts.append(xt)
                return xts

            # ---- groupnorm: packed stats across the 4 channel chunks ----
            def emit_norm(b, xts):
                st48 = stp.tile([128, 48], F32, tag="st48", name=f"st48_{b}")
                for ck in range(CK):
                    nc.vector.bn_stats(
                        st48[:, 12 * ck : 12 * ck + 6], xts[ck][:, 0:512]
                    )
                    nc.vector.bn_stats(
                        st48[:, 12 * ck + 6 : 12 * ck + 12], xts[ck][:, 512:1024]
                    )
                # view [p, c, f, t]: c=4 chunks, f=4 subgroups(256 tok), t=(cnt,mean,M2)
                st4 = st48[:, :].rearrange("p (c f t) -> p c f t", c=4, f=4, t=3)
                sum_m = stp.tile([128, CK, 1], F32, tag="sum_m")
                nc.vector.tensor_reduce(
                    sum_m[:], st4[:, :, :, 1], mybir.AxisListType.X, ALU.add
                )
                msq = stp.tile([128, CK, 4], F32, tag="msq")
                nc.vector.tensor_mul(msq[:], st4[:, :, :, 1], st4[:, :, :, 1])
                sum_m2 = stp.tile([128, CK, 1], F32, tag="sum_m2")
                nc.vector.tensor_reduce(
                    sum_m2[:], msq[:], mybir.AxisListType.X, ALU.add
                )
                sum_cv = stp.tile([128, CK, 1], F32, tag="sum_cv")
                nc.vector.tensor_reduce(
                    sum_cv[:], st4[:, :, :, 2], mybir.AxisListType.X, ALU.add
                )
                # sg8 packed [p, (c s)]: s0=sum(x), s1=sum(x^2) per channel
                sg = stp.tile([128, 2 * CK], F32, tag="sg", name=f"sg{b}")
                sgv = sg[:, :].rearrange("p (c s) -> p c s", s=2)
                nc.vector.tensor_scalar(
                    sgv[:, :, 0:1], sum_m[:], 256.0, None, ALU.mult
                )
                nc.vector.scalar_tensor_tensor(
                    sgv[:, :, 1:2], sum_m2[:], 256.0, sum_cv[:], ALU.mult, ALU.add
                )
                # XOR-butterfly partition all-reduce within each 16-channel group
                for k in (1, 2, 4, 8):
                    tmp = stp.tile([128, 2 * CK], F32, tag="shuf")
                    nc.vector.stream_shuffle(
                        tmp[:], sg[:], [i ^ k for i in range(32)]
                    )
                    sg2 = stp.tile([128, 2 * CK], F32, tag="sg", name=f"sg{b}_{k}")
                    nc.vector.tensor_add(sg2[:], sg[:], tmp[:])
                    sg = sg2
                sgv = sg[:, :].rearrange("p (c s) -> p c s", s=2)
                mean = stp.tile([128, CK, 1], F32, tag="mean")
                nc.vector.tensor_scalar(
                    mean[:], sgv[:, :, 0:1], 1.0 / 16384.0, None, ALU.mult
                )
                e2 = stp.tile([128, CK, 1], F32, tag="e2")
                nc.vector.tensor_scalar(
                    e2[:], sgv[:, :, 1:2], 1.0 / 16384.0, None, ALU.mult
                )
                msq2 = stp.tile([128, CK, 1], F32, tag="msq2")
                nc.vector.tensor_mul(msq2[:], mean[:], mean[:])
                var = stp.tile([128, CK], F32, tag="var")
                varv = var[:, :].rearrange("p (c o) -> p c o", o=1)
                nc.vector.tensor_sub(varv[:], e2[:], msq2[:])
                # rstd = exp(-0.5*ln(var+eps)) -- keeps ACT on the ln/exp table set
                lnv = stp.tile([128, CK], F32, tag="lnv")
                nc.scalar.activation(lnv[:], var[:], FX.Ln, bias=eps128[:, 0:1])
                rstd = stp.tile([128, CK], F32, tag="rstd")
                nc.scalar.activation(rstd[:], lnv[:], FX.Exp, scale=-0.5)
                svec = stp.tile([128, CK], F32, tag="svec", name=f"svec{b}")
                nc.vector.tensor_mul(svec[:], nwv[:], rstd[:])
                tms = stp.tile([128, CK], F32, tag="tms")
                nc.vector.tensor_mul(tms[:], mean[:, :, 0], svec[:])
                tvec = stp.tile([128, CK], F32, tag="tvec", name=f"tvec{b}")
                nc.vector.tensor_sub(tvec[:], nbv[:], tms[:])
                hs = [
                    hpool.tile([128, 2, N], F8, tag="h", name=f"h{b}_{pr}")
                    for pr in range(2)
                ]
                for ck in range(CK):
                    dst = hs[ck // 2][:, ck % 2, :]
                    if "no_norm" in ab:
                        nc.vector.tensor_copy(dst, xts[ck][:])
                    else:
                        nc.vector.tensor_scalar(
                            dst, xts[ck][:], svec[:, ck : ck + 1],
                            tvec[:, ck : ck + 1], ALU.mult, ALU.add,
                        )
                return hs

            def emit_qkv(b, hs):
                """QK as [d,1024] per head-slice tile; V as [128,(nt-pair,C)].
                All matmuls fp8 DoubleRow over ck-pairs (contraction 256/MM)."""
                qk = []
                for t8 in range(8):
                    ps = psb.tile([128, 1024], F32, tag="ps", name=f"qkps{b}_{t8}")
                    for qh in range(2):
                        for pr in range(2):
                            nc.tensor.matmul(
                                ps[:, 512 * qh : 512 * (qh + 1)],
                                wq8[pr][:, :, 128 * t8 : 128 * (t8 + 1)],
                                hs[pr][:, :, 512 * qh : 512 * (qh + 1)],
                                start=(pr == 0),
                                stop=(pr == 1),
                                perf_mode=DR,
                            )
                    sb = qkpool.tile([128, N], F16, tag="qk")
                    nc.scalar.activation(
                        sb[:], ps[:], FX.Identity, bias=qbv[:, t8 : t8 + 1]
                    )
                    qk.append(sb)
                vts = []
                for np_ in range(NT // 2):
                    ps = psb.tile([128, 1024], F32, tag="ps", name=f"vps{b}_{np_}")
                    for half in range(2):
                        nt = 2 * np_ + half
                        for pr in range(2):
                            nc.tensor.matmul(
                                ps[:, 512 * half : 512 * (half + 1)],
                                hs[pr][:, :, 128 * nt : 128 * (nt + 1)],
                                wq8[pr][:, :, 2 * C : 3 * C],
                                start=(pr == 0),
                                stop=(pr == 1),
                                perf_mode=DR,
                            )
                    vt = vpool.tile([128, 2, C], F8, tag="v")
                    nc.vector.tensor_copy(
                        vt[:, :, :].rearrange("p a b -> p (a b)"), ps[:]
                    )
                    vts.append(vt)
                return qk, vts

            # attention unit = (batch, head) over all 1024 queries.
            # prev unit's O/dn matmuls interleave with this unit's S/exp stream.
            # O and dn are fp8 DoubleRow over kt-pairs (V and P are pair tiles).
            def emit_prev_mms(prev, t, vts_by_b):
                pb_, ph, pptiles, dn_ps, ot_ps = prev
                NP2 = NT // 2
                for qh in range(2):
                    nc.tensor.matmul(
                        ot_ps[:, 512 * qh : 512 * (qh + 1)],
                        vts_by_b[pb_][t][:, :, 128 * ph : 128 * (ph + 1)],
                        pptiles[t][:, :, 512 * qh : 512 * (qh + 1)],
                        start=(t == 0),
                        stop=(t == NP2 - 1),
                        perf_mode=DR,
                    )
                if "no_dn" not in ab:
                    for qh in range(2):
                        nc.tensor.matmul(
                            dn_ps[:, 512 * qh : 512 * (qh + 1)],
                            ones8[:, :, :],
                            pptiles[t][:, :, 512 * qh : 512 * (qh + 1)],
                            start=(t == 0),
                            stop=(t == NP2 - 1),
                            perf_mode=DR,
                        )

            ESC = SCALE / (WSCALE * WSCALE)
            # Schraudolph exp evaluated straight into fp8e4 BITS on DVE:
            # uint8(round(log2(e^x) * 8 + 56)) IS the fp8 encoding of ~e^x,
            # at fp8's own quantization granularity. One tensor_scalar per
            # tile offloads the ACT engine (the attention-phase bottleneck);
            # the denominator (summed from the same quantized P) absorbs the
            # sawtooth error (numpy-validated: rel err unchanged). Applied to
            # the LAST kt pair only -- those P tiles are consumed latest by
            # the next unit's O/dn matmuls, so the DVE op is off the PE path.
            if "no_schr" in ab:
                SCHR_KT = ()
            elif "schr4" in ab:
                SCHR_KT = (4, 5, 6, 7)
            elif "schr3" in ab:
                SCHR_KT = (5, 6, 7)
            else:
                SCHR_KT = (6, 7)
            SCHR_A8 = 8.0 * ESC / math.log(2.0)
            SCHR_B8 = 56.0 - 8.0 * ESHIFT / math.log(2.0)

            def emit_unit(b, h, qk_by_b, vts_by_b, prev):
                """Returns (cur, prev_with_psums): prev gets its dn/ot PSUM
                tiles allocated here so its matmuls interleave with cur's."""
                q_sb, k_sb = qk_by_b[b][h], qk_by_b[b][HEADS + h]
                ptiles = [
                    ppool.tile([128, 2, 1024], F8, tag="p", name=f"p{b}_{h}_{t}")
                    for t in range(NT // 2)
                ]
                if prev is not None:
                    dn_ps = None
                    if "no_dn" not in ab:
                        dn_ps = psb.tile([128, 1024], F32, tag="ps", name=f"dn{b}_{h}")
                    ot_ps = psb.tile([128, 1024], F32, tag="ps", name=f"ot{b}_{h}")
                    prev = prev[:3] + (dn_ps, ot_ps)
                for kt in range(NT):
                    if prev is not None and kt % 2 == 0:
                        emit_prev_mms(prev, kt // 2, vts_by_b)
                    s_ps = psb.tile(
                        [128, 1024], F32, tag="ps", name=f"s{b}_{h}_{kt}"
                    )
                    for qh in range(2):
                        nc.tensor.matmul(
                            s_ps[:, 512 * qh : 512 * (qh + 1)],
                            k_sb[:, 128 * kt : 128 * (kt + 1)],
                            q_sb[:, 512 * qh : 512 * (qh + 1)],
                            start=True,
                            stop=True,
                        )
                    dst = ptiles[kt // 2][:, kt % 2, :]
                    if kt in SCHR_KT:
                        nc.vector.tensor_scalar(
                            dst.bitcast(U8), s_ps[:], SCHR_A8, SCHR_B8,
                            ALU.mult, ALU.add,
                        )
                    else:
                        nc.scalar.activation(
                            dst, s_ps[:], FX.Exp, bias=eshift[:, 0:1], scale=ESC
                        )
                return (b, h, ptiles, None, None), prev

            def emit_unit_tail(prev, osb_by_b):
                pb_, ph, pptiles, dn_ps, ot_ps = prev
                osb = osb_by_b[pb_][ph]
                if "no_dn" in ab:
                    nc.vector.tensor_copy(osb[:], ot_ps[:])
                    return
                # dn_ps rows all hold 16*denom (ones8=16 replicated matmul);
                # reciprocal folds the 1/16 v-scale compensation for free
                R_sb = spool.tile([128, 1024], F16, tag="Rsb")
                with nc.allow_low_precision(
                    reason="r in [1e-4,1e-2]: fp16 normal range, 0.1% rel err"
                ):
                    nc.vector.reciprocal(R_sb[:], dn_ps[:])
                nc.vector.tensor_mul(osb[:], ot_ps[:], R_sb[:])

            def emit_attn(qk_by_b, vts_by_b):
                osb_by_b = {
                    bb: [
                        opool.tile([128, N], F16, tag="o", name=f"osb{bb}_{i}")
                        for i in range(HEADS)
                    ]
                    for bb in (0, 1)
                }
                units = []
                for h in range(HEADS):
                    units.append((0, h))
                    units.append((1, h))
                prev = None
                pending = []
                for bb, h in units:
                    cur, prev_upd = emit_unit(bb, h, qk_by_b, vts_by_b, prev)
                    if prev_upd is not None:
                        pending.append(prev_upd)
                    while len(pending) > 1:
                        emit_unit_tail(pending.pop(0), osb_by_b)
                    prev = cur
                # last unit's O/dn matmuls (its tail is emitted by schedule()
                # between the two proj phases so proj(0) overlaps it)
                if prev is not None:
                    if "no_dn" not in ab:
                        dn_ps = psb.tile([128, 1024], F32, tag="ps", name="dnF")
                    else:
                        dn_ps = None
                    ot_ps = psb.tile([128, 1024], F32, tag="ps", name="otF")
                    prev = prev[:3] + (dn_ps, ot_ps)
                if pending:
                    emit_unit_tail(pending.pop(0), osb_by_b)
                if prev is not None:
                    for t in range(NT // 2):
                        emit_prev_mms(prev, t, vts_by_b)
                return osb_by_b, prev

            def emit_proj(b, osb, xts, t4s=range(CK)):
                for t4 in t4s:
                    ps = psb.tile([128, 1024], F32, tag="ps", name=f"prps{b}_{t4}")
                    for qh in range(2):
                        for hh in range(HEADS):
                            nc.tensor.matmul(
                                ps[:, 512 * qh : 512 * (qh + 1)],
                                wp[hh][:, 128 * t4 : 128 * (t4 + 1)],
                                osb[hh][:, 512 * qh : 512 * (qh + 1)],
                                start=(hh == 0),
                                stop=(hh == HEADS - 1),
                            )
                    yt = ypool.tile([128, N], F32, tag="y")
                    nc.vector.scalar_tensor_tensor(
                        yt[:], ps[:], pbv[:, t4 : t4 + 1], xts[t4][:],
                        ALU.add, ALU.add,
                    )
                    nc.sync.dma_start(
                        y_d.ap()[b, 128 * t4 : 128 * (t4 + 1), :], yt[:]
                    )

            # ---- schedule ----
            def schedule():
                xts0 = emit_x_dmas(0)
                hs0 = emit_norm(0, xts0)
                qk0, vts0 = emit_qkv(0, hs0)
                xts1 = emit_x_dmas(1)
                hs1 = emit_norm(1, xts1)
                qk1, vts1 = emit_qkv(1, hs1)
                vts_by_b = {0: vts0, 1: vts1}
                qk_by_b = {0: qk0, 1: qk1}
                osb_by_b, last = emit_attn(qk_by_b, vts_by_b)
                # proj(0) halves straddle the last tail: its later PSUM
                # allocations would otherwise rotate into the flush pair's
                # banks and stall until the tail frees them
                emit_proj(0, osb_by_b[0], xts0, range(0, 2))
                if last is not None:
                    emit_unit_tail(last, osb_by_b)
                emit_proj(0, osb_by_b[0], xts0, range(2, CK))
                emit_proj(1, osb_by_b[1], xts1)

            if loop_n is None:
                schedule()
            elif loop_n < 0:
                # python-unrolled repeat (for TimelineSim steady-state runs)
                for _ in range(-loop_n):
                    schedule()
            else:
                with tc.For_i(0, loop_n, 1):
                    schedule()

    _split_multiwait(nc)
    return nc


_CACHE = {}


def _get_program(loop_n=None, ablate=()):
    key = ("nc", loop_n, tuple(sorted(ablate)))
    if key not in _CACHE:
        _CACHE[key] = _build(loop_n, ablate)
    return _CACHE[key]


def _make_in_maps(inputs):
    x = np.ascontiguousarray(np.asarray(inputs["x"], dtype=np.float32))
    qkv_w = np.asarray(inputs["qkv_w"], dtype=np.float32)
    qkv_b = np.ascontiguousarray(np.asarray(inputs["qkv_b"], dtype=np.float32))
    proj_w = np.asarray(inputs["proj_w"], dtype=np.float32)
    proj_b = np.ascontiguousarray(np.asarray(inputs["proj_b"], dtype=np.float32))
    norm_w = np.ascontiguousarray(np.asarray(inputs["norm_w"], dtype=np.float32))
    norm_b = np.ascontiguousarray(np.asarray(inputs["norm_b"], dtype=np.float32))
    import ml_dtypes

    # qkv weights: x16 scale, fp8e4, DoubleRow pair layout [pair, ki, ko, out]
    qwT = qkv_w.T * WSCALE                      # [C, 3C]
    qw8 = np.ascontiguousarray(
        qwT.reshape(2, 2, 128, 3 * C).transpose(0, 2, 1, 3)
    ).astype(ml_dtypes.float8_e4m3)
    pwT = np.ascontiguousarray(proj_w.T.astype(np.float16))
    # v-bias folds through proj (softmax rows sum to 1): pb_eff = pb + Wp @ bv
    pb_eff = np.ascontiguousarray(
        proj_b + proj_w @ qkv_b[2 * C : 3 * C]
    ).astype(np.float32)
    # q/k biases ride the x16 weight scale (v-bias slot unused on device)
    qb_s = qkv_b.copy()
    qb_s[: 2 * C] *= WSCALE
    xs = x.reshape(NCORES, NB, C, N)
    in_maps = []
    for i in range(NCORES):
        in_maps.append(
            {
                "x": np.ascontiguousarray(xs[i]),
                "qw8": qw8.view(np.uint8),
                "qb": qb_s,
                "pwT": pwT,
                "pb": pb_eff,
                "nw": norm_w,
                "nb": norm_b,
            }
        )
    return in_maps


def _run(inputs, trace=False, loop_n=None, ablate=()):
    nc = _get_program(loop_n, ablate)
    in_maps = _make_in_maps(inputs)
    res = run_bass_kernel_spmd(
        nc, in_maps, core_ids=list(range(NCORES)), trace=trace
    )
    y = np.stack([res.results[i]["y"] for i in range(NCORES)], axis=0)
    y = y.reshape(B, C, H, W)
    return y, res


def kernel(**inputs) -> np.ndarray:
    y, _ = _run(inputs, trace=False)
    return y



# revision 5
# speedup vs baseline: 1.0646x; 1.0286x over previous
"""AttentionBlock (GroupNorm + 4-head self-attention + proj + residual) on 8 TRN2 cores.

Data-parallel over batch: 16 batch elements -> 2 per NeuronCore; no collectives.

x is loaded as bf16 (A/B-verified -6us/iter, high-R paired bench): the
residual tolerance (2e-2 rel ~ 0.105 abs) dwarfs bf16's ~0.2% step, and
halving x's DMA bytes shortens the chain-critical prologue (stats cannot
start until x lands). rel err 6.1e-3 vs 5.8e-3 with f32 x.

Design (per core, per batch element; C=512 channels, N=1024 tokens):
  - GroupNorm: bn_stats chains packed across all 4 channel chunks (one
    XOR-butterfly partition all-reduce on [128,8]); rstd = exp(-0.5*ln(var+eps))
    so the ACT engine only ever loads the natural_log_exp table set (no
    sqrt<->exp table reloads anywhere in the kernel); h is written straight to
    fp8e4 DoubleRow pair tiles [128, 2, 1024].
  - QKV: fp8e4 DoubleRow matmuls (contraction 256/matmul) with host-packed
    x16-scaled weight pairs; Q,K cast to fp16 [d,1024] tiles by ACT (bias
    fused); V written as [128, 2(nt-pair), 512] fp8 pair tiles by one DVE copy
    per psum pair. V-bias is folded into the proj bias on host (softmax rows
    sum to 1, so o = P(v+b) = Pv + b and proj(o+b) = proj(o) + W_p @ b).
  - Attention unit = (batch, head) over the full 1024-query range: S^T = K^T Q
    in fp16 (no PE transposes; two query-halves share a 2-bank PSUM tile; ONE
    exp per kt covers [128,1024]); softmax max-shift replaced by a constant
    shift (exp(s*scale - 2.5), inputs are bounded so this cannot overflow and
    keeps P inside fp8e4's +-240); P written directly to fp8 DoubleRow pair
    tiles. O^T = V P and the denominator run as fp8 DoubleRow over kt-pairs,
    interleaved with the next unit's S stream to keep PE dense. The
    denominator matmul uses lhsT = 16*ones[128,2,128], replicating 16*denom to
    all 128 PSUM rows: the tail is then just DVE reciprocal (which also folds
    the 1/16 weight-scale compensation) + one multiply -- no partition
    broadcast needed. Tails are deferred one unit.
  - proj: plain fp16 matmuls over fp16 O tiles; residual + bias fused in one
    scalar_tensor_tensor; x stays resident in SBUF from the norm phase (no
    reload for the residual). proj(0) is emitted in two halves straddling the
    last unit's tail so its PSUM allocations don't rotate into (and stall on)
    the flush pair's banks.

Environment workaround: this walrus build encodes at most one semaphore wait
per instruction; _split_multiwait() moves excess waits onto injected
same-engine NoOps after TileContext scheduling.
"""

import math

import numpy as np

import concourse.bass as bass
import concourse.mybir as mybir
from concourse import tile
from concourse.bass_utils import run_bass_kernel_spmd

# problem constants (self-contained by contract)
B, C, H, W = 16, 512, 32, 32
N = H * W
HEADS, D = 4, 128
G = 32
EPS = 1e-5
SCALE = 1.0 / math.sqrt(D)
NCORES = 8
NB = B // NCORES  # batch elems per core
CK = C // 128     # channel chunks
NT = N // 128     # token tiles
F32 = mybir.dt.float32
F16 = mybir.dt.float16
BF16 = mybir.dt.bfloat16
F8 = mybir.dt.float8e4
I32 = mybir.dt.int32
U8 = mybir.dt.uint8
FX = mybir.ActivationFunctionType
ALU = mybir.AluOpType
DR = mybir.MatmulPerfMode.DoubleRow

WARM_MMS = 24  # PE warmup matmuls issued during the groupnorm prologue
WSCALE = 16.0   # qkv weights are host-scaled by 16 for fp8e4 range coverage
ESHIFT = 2.5    # global exp shift: keeps P = exp(s - ESHIFT) under fp8e4 +-240


# --- workaround: this walrus encodes at most ONE sync wait per instruction ---
_waitctr = [0]


def _split_multiwait(nc):
    for fn in nc.m.functions:
        for bb in fn.blocks:
            out = []
            changed = False
            for inst in bb.instructions:
                si = inst.sync_info
                if si is not None and len(si.on_wait) > 1:
                    waits = list(si.on_wait)
                    for wt in waits[:-1]:
                        _waitctr[0] += 1
                        nop = mybir.InstNoOp(
                            name=f"I-waitsplit-{_waitctr[0]}", ins=[], outs=[]
                        )
                        nop.engine = inst.engine
                        nop.sync_info = mybir.SyncInfo(on_wait=[wt], on_update=[])
                        out.append(nop)
                    inst.sync_info = mybir.SyncInfo(
                        on_wait=[waits[-1]], on_update=list(si.on_update)
                    )
                    changed = True
                out.append(inst)
            if changed:
                bb.instructions = out


def _build(loop_n=None, ablate=()):
    ab = set(ablate)
    nc = bass.Bass("TRN2", target_bir_lowering=False, debug=False)

    x_d = nc.dram_tensor("x", [NB, C, N], BF16, kind="ExternalInput")
    qw8_d = nc.dram_tensor("qw8", [2, 128, 2, 3 * C], F8, kind="ExternalInput")
    qb_d = nc.dram_tensor("qb", [3 * C], F32, kind="ExternalInput")
    pwT_d = nc.dram_tensor("pwT", [C, C], F16, kind="ExternalInput")
    pb_d = nc.dram_tensor("pb", [C], F32, kind="ExternalInput")
    nw_d = nc.dram_tensor("nw", [C], F32, kind="ExternalInput")
    nb_d = nc.dram_tensor("nb", [C], F32, kind="ExternalInput")
    y_d = nc.dram_tensor("y", [NB, C, N], F32, kind="ExternalOutput")

    with tile.TileContext(nc) as tc:
        import contextlib

        with contextlib.ExitStack() as ctx:
            wpool = ctx.enter_context(tc.tile_pool(name="wpool", bufs=1))
            xpool = ctx.enter_context(tc.tile_pool(name="xpool", bufs=8))
            spool = ctx.enter_context(tc.tile_pool(name="spool", bufs=3))
            stp = ctx.enter_context(tc.tile_pool(name="stp", bufs=3))
            hpool = ctx.enter_context(tc.tile_pool(name="hpool", bufs=8))
            qkpool = ctx.enter_context(tc.tile_pool(name="qkpool", bufs=17))
            vpool = ctx.enter_context(tc.tile_pool(name="vpool", bufs=9))
            ppool = ctx.enter_context(tc.tile_pool(name="ppool", bufs=9))
            opool = ctx.enter_context(tc.tile_pool(name="opool", bufs=9))
            ypool = ctx.enter_context(tc.tile_pool(name="ypool", bufs=2))
            psb = ctx.enter_context(tc.tile_pool(name="psb", bufs=4, space="PSUM"))

            # ---- constants & weights ----
            ones_col = wpool.tile([128, 1], F16, tag="ones_col")
            nc.gpsimd.memset(ones_col[:], 1.0)
            # dn lhsT: pair-axis step must be 16B-aligned (s3_lw dual-fp8 rule)
            # dn lhsT: 16*ones, M=128 -> dn replicated to all PSUM rows; the 16
            # also folds the 1/WSCALE v-compensation into the reciprocal.
            ones8 = wpool.tile([128, 2, 128], F8, tag="ones8")
            nc.gpsimd.memset(ones8[:], WSCALE)
            ones_w = wpool.tile([128, 512], F16, tag="ones_w")
            nc.gpsimd.memset(ones_w[:], 1.0)
            eps128 = wpool.tile([128, 1], F32, tag="eps")
            nc.gpsimd.memset(eps128[:], EPS)
            eshift = wpool.tile([128, 1], F32, tag="eshift")
            nc.gpsimd.memset(eshift[:], -ESHIFT)

            # PE warmup (opt-in): drift-free A/B showed the warmup matmuls
            # cost ~30us/iter in the repeat-loop regime, so default is off.
            warm_ps = psb.tile([128, 1024], F32, tag="ps", name="warmps")
            for _ in range(WARM_MMS if "warm" in ab else 0):
                nc.tensor.matmul(
                    warm_ps[0:1, 0:512], ones_col[:], ones_w[:], start=True, stop=True
                )

            wq8 = []
            for pr in range(2):
                t = wpool.tile([128, 2, 3 * C], F8, tag=f"wq8_{pr}")
                nc.sync.dma_start(t[:], qw8_d.ap()[pr, :, :, :])
                wq8.append(t)
            wp = []
            for ck in range(CK):
                t = wpool.tile([128, C], F16, tag=f"wp{ck}")
                nc.sync.dma_start(t[:], pwT_d.ap()[128 * ck : 128 * (ck + 1), :])
                wp.append(t)
            qbv = wpool.tile([128, 12], F32, tag="qbv")
            nc.sync.dma_start(qbv[:], qb_d.ap()[:].rearrange("(c p) -> p c", p=128))
            pbv = wpool.tile([128, CK], F32, tag="pbv")
            nc.sync.dma_start(pbv[:], pb_d.ap()[:].rearrange("(c p) -> p c", p=128))
            nwv = wpool.tile([128, CK], F32, tag="nwv")
            nc.sync.dma_start(nwv[:], nw_d.ap()[:].rearrange("(c p) -> p c", p=128))
            nbv = wpool.tile([128, CK], F32, tag="nbv")
            nc.sync.dma_start(nbv[:], nb_d.ap()[:].rearrange("(c p) -> p c", p=128))

            # x loads split per token-half so bn_stats can start at half-tile
            def emit_x_dmas(b):
                xts = []
                for ck in range(CK):
                    xt = xpool.tile([128, N], BF16, tag="x", name=f"xt{b}_{ck}")
                    for hf in range(2):
                        nc.sync.dma_start(
                            xt[:, 512 * hf : 512 * (hf + 1)],
                            x_d.ap()[
                                b, 128 * ck : 128 * (ck + 1),
                                512 * hf : 512 * (hf + 1),
                            ],
                        )
                    xts.append(xt)
                return xts

            # ---- groupnorm: packed stats across the 4 channel chunks ----
            def emit_norm(b, xts):
                st48 = stp.tile([128, 48], F32, tag="st48", name=f"st48_{b}")
                for ck in range(CK):
                    nc.vector.bn_stats(
                        st48[:, 12 * ck : 12 * ck + 6], xts[ck][:, 0:512]
                    )
                    nc.vector.bn_stats(
                        st48[:, 12 * ck + 6 : 12 * ck + 12], xts[ck][:, 512:1024]
                    )
                # view [p, c, f, t]: c=4 chunks, f=4 subgroups(256 tok), t=(cnt,mean,M2)
                st4 = st48[:, :].rearrange("p (c f t) -> p c f t", c=4, f=4, t=3)
                sum_m = stp.tile([128, CK, 1], F32, tag="sum_m")
                nc.vector.tensor_reduce(
                    sum_m[:], st4[:, :, :, 1], mybir.AxisListType.X, ALU.add
                )
                msq = stp.tile([128, CK, 4], F32, tag="msq")
                nc.vector.tensor_mul(msq[:], st4[:, :, :, 1], st4[:, :, :, 1])
                sum_m2 = stp.tile([128, CK, 1], F32, tag="sum_m2")
                nc.vector.tensor_reduce(
                    sum_m2[:], msq[:], mybir.AxisListType.X, ALU.add
                )
                sum_cv = stp.tile([128, CK, 1], F32, tag="sum_cv")
                nc.vector.tensor_reduce(
                    sum_cv[:], st4[:, :, :, 2], mybir.AxisListType.X, ALU.add
                )
                # sg8 packed [p, (c s)]: s0=sum(x), s1=sum(x^2) per channel
                sg = stp.tile([128, 2 * CK], F32, tag="sg", name=f"sg{b}")
                sgv = sg[:, :].rearrange("p (c s) -> p c s", s=2)
                nc.vector.tensor_scalar(
                    sgv[:, :, 0:1], sum_m[:], 256.0, None, ALU.mult
                )
                nc.vector.scalar_tensor_tensor(
                    sgv[:, :, 1:2], sum_m2[:], 256.0, sum_cv[:], ALU.mult, ALU.add
                )
                # XOR-butterfly partition all-reduce within each 16-channel group
                for k in (1, 2, 4, 8):
                    tmp = stp.tile([128, 2 * CK], F32, tag="shuf")
                    nc.vector.stream_shuffle(
                        tmp[:], sg[:], [i ^ k for i in range(32)]
                    )
                    sg2 = stp.tile([128, 2 * CK], F32, tag="sg", name=f"sg{b}_{k}")
                    nc.vector.tensor_add(sg2[:], sg[:], tmp[:])
                    sg = sg2
                sgv = sg[:, :].rearrange("p (c s) -> p c s", s=2)
                mean = stp.tile([128, CK, 1], F32, tag="mean")
                nc.vector.tensor_scalar(
                    mean[:], sgv[:, :, 0:1], 1.0 / 16384.0, None, ALU.mult
                )
                e2 = stp.tile([128, CK, 1], F32, tag="e2")
                nc.vector.tensor_scalar(
                    e2[:], sgv[:, :, 1:2], 1.0 / 16384.0, None, ALU.mult
                )
                msq2 = stp.tile([128, CK, 1], F32, tag="msq2")
                nc.vector.tensor_mul(msq2[:], mean[:], mean[:])
                var = stp.tile([128, CK], F32, tag="var")
                varv = var[:, :].rearrange("p (c o) -> p c o", o=1)
                nc.vector.tensor_sub(varv[:], e2[:], msq2[:])
                # rstd = exp(-0.5*ln(var+eps)) -- keeps ACT on the ln/exp table set
                lnv = stp.tile([128, CK], F32, tag="lnv")
                nc.scalar.activation(lnv[:], var[:], FX.Ln, bias=eps128[:, 0:1])
                rstd = stp.tile([128, CK], F32, tag="rstd")
                nc.scalar.activation(rstd[:], lnv[:], FX.Exp, scale=-0.5)
                svec = stp.tile([128, CK], F32, tag="svec", name=f"svec{b}")
                nc.vector.tensor_mul(svec[:], nwv[:], rstd[:])
                tms = stp.tile([128, CK], F32, tag="tms")
                nc.vector.tensor_mul(tms[:], mean[:, :, 0], svec[:])
                tvec = stp.tile([128, CK], F32, tag="tvec", name=f"tvec{b}")
                nc.vector.tensor_sub(tvec[:], nbv[:], tms[:])
                hs = [
                    hpool.tile([128, 2, N], F8, tag="h", name=f"h{b}_{pr}")
                    for pr in range(2)
                ]
                for ck in range(CK):
                    dst = hs[ck // 2][:, ck % 2, :]
                    if "no_norm" in ab:
                        nc.vector.tensor_copy(dst, xts[ck][:])
                    else:
                        nc.vector.tensor_scalar(
                            dst, xts[ck][:], svec[:, ck : ck + 1],
                            tvec[:, ck : ck + 1], ALU.mult, ALU.add,
                        )
                return hs

            def emit_qkv(b, hs):
                """QK as [d,1024] per head-slice tile; V as [128,(nt-pair,C)].
                All matmuls fp8 DoubleRow over ck-pairs (contraction 256/MM)."""
                qk = []
                for t8 in range(8):
                    ps = psb.tile([128, 1024], F32, tag="ps", name=f"qkps{b}_{t8}")
                    for qh in range(2):
                        for pr in range(2):
                            nc.tensor.matmul(
                                ps[:, 512 * qh : 512 * (qh + 1)],
                                wq8[pr][:, :, 128 * t8 : 128 * (t8 + 1)],
                                hs[pr][:, :, 512 * qh : 512 * (qh + 1)],
                                start=(pr == 0),
                                stop=(pr == 1),
                                perf_mode=DR,
                            )
                    sb = qkpool.tile([128, N], F16, tag="qk")
                    nc.scalar.activation(
                        sb[:], ps[:], FX.Identity, bias=qbv[:, t8 : t8 + 1]
                    )
                    qk.append(sb)
                vts = []
                for np_ in range(NT // 2):
                    ps = psb.tile([128, 1024], F32, tag="ps", name=f"vps{b}_{np_}")
                    for half in range(2):
                        nt = 2 * np_ + half
                        for pr in range(2):
                            nc.tensor.matmul(
                                ps[:, 512 * half : 512 * (half + 1)],
                                hs[pr][:, :, 128 * nt : 128 * (nt + 1)],
                                wq8[pr][:, :, 2 * C : 3 * C],
                                start=(pr == 0),
                                stop=(pr == 1),
                                perf_mode=DR,
                            )
                    vt = vpool.tile([128, 2, C], F8, tag="v")
                    nc.vector.tensor_copy(
                        vt[:, :, :].rearrange("p a b -> p (a b)"), ps[:]
                    )
                    vts.append(vt)
                return qk, vts

            # attention unit = (batch, head) over all 1024 queries.
            # prev unit's O/dn matmuls interleave with this unit's S/exp stream.
            # O and dn are fp8 DoubleRow over kt-pairs (V and P are pair tiles).
            def emit_prev_mms(prev, t, vts_by_b):
                pb_, ph, pptiles, dn_ps, ot_ps = prev
                NP2 = NT // 2
                for qh in range(2):
                    nc.tensor.matmul(
                        ot_ps[:, 512 * qh : 512 * (qh + 1)],
                        vts_by_b[pb_][t][:, :, 128 * ph : 128 * (ph + 1)],
                        pptiles[t][:, :, 512 * qh : 512 * (qh + 1)],
                        start=(t == 0),
                        stop=(t == NP2 - 1),
                        perf_mode=DR,
                    )
                if "no_dn" not in ab:
                    for qh in range(2):
                        nc.tensor.matmul(
                            dn_ps[:, 512 * qh : 512 * (qh + 1)],
                            ones8[:, :, :],
                            pptiles[t][:, :, 512 * qh : 512 * (qh + 1)],
                            start=(t == 0),
                            stop=(t == NP2 - 1),
                            perf_mode=DR,
                        )

            ESC = SCALE / (WSCALE * WSCALE)
            # Schraudolph exp evaluated straight into fp8e4 BITS on DVE:
            # uint8(round(log2(e^x) * 8 + 56)) IS the fp8 encoding of ~e^x,
            # at fp8's own quantization granularity. One tensor_scalar per
            # tile offloads the ACT engine (the attention-phase bottleneck);
            # the denominator (summed from the same quantized P) absorbs the
            # sawtooth error (numpy-validated: rel err unchanged). Applied to
            # the LAST kt pair only -- those P tiles are consumed latest by
            # the next unit's O/dn matmuls, so the DVE op is off the PE path.
            if "no_schr" in ab:
                SCHR_KT = ()
            elif "schr4" in ab:
                SCHR_KT = (4, 5, 6, 7)
            elif "schr3" in ab:
                SCHR_KT = (5, 6, 7)
            else:
                SCHR_KT = (6, 7)
            SCHR_A8 = 8.0 * ESC / math.log(2.0)
            SCHR_B8 = 56.0 - 8.0 * ESHIFT / math.log(2.0)

            def emit_unit(b, h, qk_by_b, vts_by_b, prev):
                """Returns (cur, prev_with_psums): prev gets its dn/ot PSUM
                tiles allocated here so its matmuls interleave with cur's."""
                q_sb, k_sb = qk_by_b[b][h], qk_by_b[b][HEADS + h]
                ptiles = [
                    ppool.tile([128, 2, 1024], F8, tag="p", name=f"p{b}_{h}_{t}")
                    for t in range(NT // 2)
                ]
                if prev is not None:
                    dn_ps = None
                    if "no_dn" not in ab:
                        dn_ps = psb.tile([128, 1024], F32, tag="ps", name=f"dn{b}_{h}")
                    ot_ps = psb.tile([128, 1024], F32, tag="ps", name=f"ot{b}_{h}")
                    prev = prev[:3] + (dn_ps, ot_ps)
                for kt in range(NT):
                    if prev is not None and kt % 2 == 0:
                        emit_prev_mms(prev, kt // 2, vts_by_b)
                    s_ps = psb.tile(
                        [128, 1024], F32, tag="ps", name=f"s{b}_{h}_{kt}"
                    )
                    for qh in range(2):
                        nc.tensor.matmul(
                            s_ps[:, 512 * qh : 512 * (qh + 1)],
                            k_sb[:, 128 * kt : 128 * (kt + 1)],
                            q_sb[:, 512 * qh : 512 * (qh + 1)],
                            start=True,
                            stop=True,
                        )
                    dst = ptiles[kt // 2][:, kt % 2, :]
                    if kt in SCHR_KT:
                        nc.vector.tensor_scalar(
                            dst.bitcast(U8), s_ps[:], SCHR_A8, SCHR_B8,
                            ALU.mult, ALU.add,
                        )
                    else:
                        nc.scalar.activation(
                            dst, s_ps[:], FX.Exp, bias=eshift[:, 0:1], scale=ESC
                        )
                return (b, h, ptiles, None, None), prev

            def emit_unit_tail(prev, osb_by_b):
                pb_, ph, pptiles, dn_ps, ot_ps = prev
                osb = osb_by_b[pb_][ph]
                if "no_dn" in ab:
                    nc.vector.tensor_copy(osb[:], ot_ps[:])
                    return
                # dn_ps rows all hold 16*denom (ones8=16 replicated matmul);
                # reciprocal folds the 1/16 v-scale compensation for free
                R_sb = spool.tile([128, 1024], F16, tag="Rsb")
                with nc.allow_low_precision(
                    reason="r in [1e-4,1e-2]: fp16 normal range, 0.1% rel err"
                ):
                    nc.vector.reciprocal(R_sb[:], dn_ps[:])
                nc.vector.tensor_mul(osb[:], ot_ps[:], R_sb[:])

            def emit_attn(qk_by_b, vts_by_b):
                osb_by_b = {
                    bb: [
                        opool.tile([128, N], F16, tag="o", name=f"osb{bb}_{i}")
                        for i in range(HEADS)
                    ]
                    for bb in (0, 1)
                }
                units = []
                for h in range(HEADS):
                    units.append((0, h))
                    units.append((1, h))
                prev = None
                pending = []
                for bb, h in units:
                    cur, prev_upd = emit_unit(bb, h, qk_by_b, vts_by_b, prev)
                    if prev_upd is not None:
                        pending.append(prev_upd)
                    while len(pending) > 1:
                        emit_unit_tail(pending.pop(0), osb_by_b)
                    prev = cur
                # last unit's O/dn matmuls (its tail is emitted by schedule()
                # between the two proj phases so proj(0) overlaps it)
                if prev is not None:
                    if "no_dn" not in ab:
                        dn_ps = psb.tile([128, 1024], F32, tag="ps", name="dnF")
                    else:
                        dn_ps = None
                    ot_ps = psb.tile([128, 1024], F32, tag="ps", name="otF")
                    prev = prev[:3] + (dn_ps, ot_ps)
                if pending:
                    emit_unit_tail(pending.pop(0), osb_by_b)
                if prev is not None:
                    for t in range(NT // 2):
                        emit_prev_mms(prev, t, vts_by_b)
                return osb_by_b, prev

            def emit_proj(b, osb, xts, t4s=range(CK)):
                for t4 in t4s:
                    ps = psb.tile([128, 1024], F32, tag="ps", name=f"prps{b}_{t4}")
                    for qh in range(2):
                        for hh in range(HEADS):
                            nc.tensor.matmul(
                                ps[:, 512 * qh : 512 * (qh + 1)],
                                wp[hh][:, 128 * t4 : 128 * (t4 + 1)],
                                osb[hh][:, 512 * qh : 512 * (qh + 1)],
                                start=(hh == 0),
                                stop=(hh == HEADS - 1),
                            )
                    yt = ypool.tile([128, N], F32, tag="y")
                    nc.vector.scalar_tensor_tensor(
                        yt[:], ps[:], pbv[:, t4 : t4 + 1], xts[t4][:],
                        ALU.add, ALU.add,
                    )
                    nc.sync.dma_start(
                        y_d.ap()[b, 128 * t4 : 128 * (t4 + 1), :], yt[:]
                    )

            # ---- schedule ----
            def schedule():
                xts0 = emit_x_dmas(0)
                hs0 = emit_norm(0, xts0)
                qk0, vts0 = emit_qkv(0, hs0)
                xts1 = emit_x_dmas(1)
                hs1 = emit_norm(1, xts1)
                qk1, vts1 = emit_qkv(1, hs1)
                vts_by_b = {0: vts0, 1: vts1}
                qk_by_b = {0: qk0, 1: qk1}
                osb_by_b, last = emit_attn(qk_by_b, vts_by_b)
                # proj(0) halves straddle the last tail: its later PSUM
                # allocations would otherwise rotate into the flush pair's
                # banks and stall until the tail frees them
                emit_proj(0, osb_by_b[0], xts0, range(0, 2))
                if last is not None:
                    emit_unit_tail(last, osb_by_b)
                emit_proj(0, osb_by_b[0], xts0, range(2, CK))
                emit_proj(1, osb_by_b[1], xts1)

            if loop_n is None:
                schedule()
            elif loop_n < 0:
                # python-unrolled repeat (for TimelineSim steady-state runs)
                for _ in range(-loop_n):
                    schedule()
            else:
                with tc.For_i(0, loop_n, 1):
                    schedule()

    _split_multiwait(nc)
    return nc


_CACHE = {}


def _get_program(loop_n=None, ablate=()):
    key = ("nc", loop_n, tuple(sorted(ablate)))
    if key not in _CACHE:
        _CACHE[key] = _build(loop_n, ablate)
    return _CACHE[key]


def _make_in_maps(inputs):
    x = np.ascontiguousarray(np.asarray(inputs["x"], dtype=np.float32))
    qkv_w = np.asarray(inputs["qkv_w"], dtype=np.float32)
    qkv_b = np.ascontiguousarray(np.asarray(inputs["qkv_b"], dtype=np.float32))
    proj_w = np.asarray(inputs["proj_w"], dtype=np.float32)
    proj_b = np.ascontiguousarray(np.asarray(inputs["proj_b"], dtype=np.float32))
    norm_w = np.ascontiguousarray(np.asarray(inputs["norm_w"], dtype=np.float32))
    norm_b = np.ascontiguousarray(np.asarray(inputs["norm_b"], dtype=np.float32))
    import ml_dtypes

    # qkv weights: x16 scale, fp8e4, DoubleRow pair layout [pair, ki, ko, out]
    qwT = qkv_w.T * WSCALE                      # [C, 3C]
    qw8 = np.ascontiguousarray(
        qwT.reshape(2, 2, 128, 3 * C).transpose(0, 2, 1, 3)
    ).astype(ml_dtypes.float8_e4m3)
    pwT = np.ascontiguousarray(proj_w.T.astype(np.float16))
    # v-bias folds through proj (softmax rows sum to 1): pb_eff = pb + Wp @ bv
    pb_eff = np.ascontiguousarray(
        proj_b + proj_w @ qkv_b[2 * C : 3 * C]
    ).astype(np.float32)
    # q/k biases ride the x16 weight scale (v-bias slot unused on device)
    qb_s = qkv_b.copy()
    qb_s[: 2 * C] *= WSCALE
    xs = x.reshape(NCORES, NB, C, N).astype(ml_dtypes.bfloat16)
    in_maps = []
    for i in range(NCORES):
        in_maps.append(
            {
                "x": np.ascontiguousarray(xs[i]).view(np.uint16),
                "qw8": qw8.view(np.uint8),
                "qb": qb_s,
                "pwT": pwT,
                "pb": pb_eff,
                "nw": norm_w,
                "nb": norm_b,
            }
        )
    return in_maps


def _run(inputs, trace=False, loop_n=None, ablate=()):
    nc = _get_program(loop_n, ablate)
    in_maps = _make_in_maps(inputs)
    res = run_bass_kernel_spmd(
        nc, in_maps, core_ids=list(range(NCORES)), trace=trace
    )
    y = np.stack([res.results[i]["y"] for i in range(NCORES)], axis=0)
    y = y.reshape(B, C, H, W)
    return y, res


def kernel(**inputs) -> np.ndarray:
    y, _ = _run(inputs, trace=False)
    return y

